# revision 35
# baseline (speedup 1.0000x reference)
"""BiMamba (bidirectional Mamba block) Trainium2 kernel.

Sharding: 8 cores = 2 (batch) x 4 (d_inner quarters of 384 channels).
Per core: in_proj (x,z slices) -> depthwise causal conv (fwd + reversed)
-> silu -> x_proj partials -> AllReduce over the 4-core d-shard group
-> delta(softplus) -> selective scan via DVE tensor_tensor_scan (channels
on partitions, state segments along the free axis) -> gating -> out_proj
partial.  Host sums the 4 partials per batch.

flip(L) commutes with L-wise matmuls, so in_proj/out_proj run once:
  out = (Y_f + flip(Y_r)) * silu(z) @ W_o^T.

This walrus build supports only ONE sync-wait per Matmult, so the kernel is
structured so every matmul carries at most one wait:
  - all PE-feeding weight tiles are DMA'd and "touched" (1x1 self-matmul
    into a scratch PSUM corner) in a preamble, putting their DMA semaphores
    into the PE's observed vector clock;
  - PSUM tiles are allocated once per tag and overwritten (no slot
    recycling), so a group's first matmul waits only on the single-engine
    WAR release of the previous group's readers;
  - stages A..E share one PSUM pool (no pool transition deps); the F and I
    pools' transition deps are absorbed by DVE memset guards, which
    coalesce with the (also DVE) data dependency of their first matmuls.
"""

import numpy as np

D_MODEL = 768
D_INNER = 1536
D_STATE = 16
D_CONV = 4
DT_RANK = 48
L = 1024
B = 2
N_CORES = 8
N_GROUPS = 4               # d-shard groups
CH = D_INNER // N_GROUPS   # 384 channels per core
CT = CH // 128             # 3 channel partition-tiles
NSEG = 2                   # state segments per scan chunk
NCHUNK = D_STATE // NSEG
XD = DT_RANK + 2 * D_STATE  # 80

_cache = {}


def _build():
    import concourse.bass as bass
    import concourse.bacc as bacc
    import concourse.mybir as mybir
    from concourse import tile

    f32 = mybir.dt.float32
    bf16 = mybir.dt.bfloat16
    AF = mybir.ActivationFunctionType
    OP = mybir.AluOpType

    nc = bacc.Bacc("TRN2", target_bir_lowering=False, debug=False,
                   num_devices=N_CORES)

    hT = nc.dram_tensor("hT", [D_MODEL, L], f32, kind="ExternalInput").ap()
    w_in = nc.dram_tensor("w_in", [D_MODEL, 2 * CH], f32, kind="ExternalInput").ap()
    consts = nc.dram_tensor("consts", [2 * CH, 23], f32, kind="ExternalInput").ap()
    xp_w = nc.dram_tensor("xp_w", [2 * CH, 80], f32, kind="ExternalInput").ap()
    dt_wT = nc.dram_tensor("dt_wT", [2 * DT_RANK, CH], f32, kind="ExternalInput").ap()
    wo = nc.dram_tensor("wo", [CH, D_MODEL], f32, kind="ExternalInput").ap()
    sel_d = nc.dram_tensor("sel", [2 * D_STATE, 2 * D_STATE * 128], bf16,
                           kind="ExternalInput").ap()
    outp = nc.dram_tensor("outp", [D_MODEL, L], f32, kind="ExternalOutput").ap()

    def rev_view(ap, n=L):
        return bass.AP(ap.tensor, ap.offset + (n - 1), [ap.ap[0], [-1, n]])

    def bcast_seg(ap, nseg):
        return bass.AP(ap.tensor, ap.offset, [ap.ap[0], [0, nseg], [1, L]])

    with tile.TileContext(nc) as tc:
        with (
            tc.tile_pool(name="persist", bufs=1) as pp,
            tc.tile_pool(name="dram", bufs=1, space="DRAM") as dp,
            tc.tile_pool(name="mid", bufs=1) as mp,
            tc.tile_pool(name="psG1", bufs=1, space="PSUM") as psG1,
        ):
            # ---------- persistent tiles + PE preamble touchers ----------
            scr = psG1.tile([1, 8], f32, tag="scr", name="scr")

            def touch(ap):
                nc.tensor.matmul(scr[0:1, 0:1], ap[:, 0:1], ap[:, 0:1],
                                 start=True, stop=True)

            sz = [pp.tile([128, L], f32, tag=f"sz{i}", name=f"sz{i}")
                  for i in range(CT)]
            cst = [pp.tile([128, 23], f32, tag=f"cst{j}", name=f"cst{j}")
                   for j in range(2 * CT)]
            for j in range(2 * CT):
                nc.sync.dma_start(cst[j][:], consts[128 * j:128 * (j + 1), :])
            touch(cst[0][:])
            selt = pp.tile([2 * D_STATE, 2 * D_STATE * 128], bf16, tag="sel",
                           name="selt")
            nc.sync.dma_start(selt[:], sel_d[:])
            touch(selt[:])
            xpw_t = []
            for j in range(2 * CT):
                w_j = pp.tile([128, 80], f32, tag=f"xpw{j}", name=f"xpw{j}")
                nc.sync.dma_start(w_j[:], xp_w[128 * j:128 * (j + 1), :])
                touch(w_j[:])
                xpw_t.append(w_j)
            dtw_t = []
            for d in range(2):
                w_d = pp.tile([DT_RANK, CH], f32, tag=f"dtw{d}",
                              name=f"dtw{d}")
                nc.sync.dma_start(w_d[:],
                                  dt_wT[DT_RANK * d:DT_RANK * (d + 1), :])
                touch(w_d[:])
                dtw_t.append(w_d)
            wo_t = []
            for k in range(CT):
                w_k = pp.tile([128, D_MODEL], f32, tag=f"wo{k}",
                              name=f"wo{k}")
                nc.sync.dma_start(w_k[:], wo[128 * k:128 * (k + 1), :])
                touch(w_k[:])
                wo_t.append(w_k)

            ar_in = dp.tile([2 * XD, L], f32, name="ar_in")
            ar_out = dp.tile([2 * XD, L], f32, name="ar_out")

            if True:
                with tc.tile_pool(name="xtp", bufs=1) as xp_pool:
                    xt = [xp_pool.tile([128, L], f32, tag=f"xt{j}",
                                       name=f"xt{j}") for j in range(2 * CT)]

                    # ------- Stage A: in_proj; Stage B: conv + silu -------
                    with tc.tile_pool(name="stab", bufs=1) as ab:
                        ps_in = psG1.tile([128, 512], f32, tag="ps_in",
                                          name="ps_in")
                        xpad = [ab.tile([128, L + D_CONV - 1], f32,
                                        tag=f"xp{i}", name=f"xp{i}")
                                for i in range(CT)]
                        xrpad = [ab.tile([128, L + D_CONV - 1], f32,
                                         tag=f"xr{i}", name=f"xr{i}")
                                 for i in range(CT)]
                        for i in range(CT):
                            nc.gpsimd.memset(xpad[i][:, 0:D_CONV - 1], 0.0)
                            nc.gpsimd.memset(xrpad[i][:, 0:D_CONV - 1], 0.0)
                        for f in range(2):
                            hTc = []
                            for k in range(6):
                                h_k = ab.tile([128, 512], f32, tag=f"hTc{k}",
                                              name=f"hTc{f}_{k}")
                                nc.sync.dma_start(
                                    h_k[:], hT[128 * k:128 * (k + 1),
                                               512 * f:512 * (f + 1)])
                                touch(h_k[:])
                                hTc.append(h_k)
                            for o in range(6):
                                win_t = []
                                for k in range(6):
                                    w_k = ab.tile(
                                        [128, 128], f32, tag=f"win{k}",
                                        bufs=2, name=f"win{f}_{o}_{k}")
                                    nc.sync.dma_start(
                                        w_k[:], w_in[128 * k:128 * (k + 1),
                                                     128 * o:128 * (o + 1)])
                                    touch(w_k[:])
                                    win_t.append(w_k)
                                ps = ps_in
                                for k in range(6):
                                    nc.tensor.matmul(ps[:], win_t[k][:],
                                                     hTc[k][:],
                                                     start=(k == 0),
                                                     stop=(k == 5))
                                if o < CT:
                                    pa = D_CONV - 1
                                    dst = xpad[o][:, pa + 512 * f:
                                                  pa + 512 * (f + 1)]
                                    nc.scalar.activation(dst, ps[:], AF.Copy)
                                    nc.vector.tensor_copy(
                                        xrpad[o][:, pa + 512 * (1 - f):
                                                 pa + 512 * (2 - f)],
                                        rev_view(dst, 512))
                                else:
                                    i = o - CT
                                    sl = slice(512 * f, 512 * (f + 1))
                                    zc = ab.tile([128, 512], f32,
                                                 tag="cacc",
                                                 bufs=2, name=f"zc{f}_{o}")
                                    nc.scalar.activation(zc[:], ps[:],
                                                         AF.Copy)
                                    nc.scalar.activation(sz[i][:, sl], ps[:],
                                                         AF.Sigmoid)
                                    nc.vector.tensor_mul(sz[i][:, sl],
                                                         sz[i][:, sl], zc[:])

                        for d in range(2):
                            src = xpad if d == 0 else xrpad
                            for i in range(CT):
                                c = cst[d * CT + i]
                                for hh in range(2):
                                    c0 = 512 * hh
                                    acc = ab.tile([128, 512], f32,
                                                  tag="cacc", bufs=2,
                                                  name=f"cacc{d}_{i}_{hh}")
                                    nc.vector.scalar_tensor_tensor(
                                        acc[:], src[i][:, c0:c0 + 512],
                                        c[:, 0:1], src[i][:, c0:c0 + 512],
                                        op0=OP.mult, op1=OP.bypass)
                                    for k in range(1, D_CONV):
                                        nc.vector.scalar_tensor_tensor(
                                            acc[:],
                                            src[i][:, c0 + k:c0 + k + 512],
                                            c[:, k:k + 1], acc[:],
                                            op0=OP.mult, op1=OP.add)
                                    xb = ab.tile([128, 512], f32, tag="xb",
                                                 bufs=2,
                                                 name=f"xb{d}_{i}_{hh}")
                                    sg = ab.tile([128, 512], f32, tag="sg",
                                                 bufs=2,
                                                 name=f"sg{d}_{i}_{hh}")
                                    nc.scalar.activation(xb[:], acc[:],
                                                         AF.Identity,
                                                         bias=c[:, 4:5])
                                    nc.scalar.activation(sg[:], acc[:],
                                                         AF.Sigmoid,
                                                         bias=c[:, 4:5])
                                    nc.vector.tensor_mul(
                                        xt[d * CT + i][:, c0:c0 + 512],
                                        sg[:], xb[:])

                    # ------- Stage C: x_proj partials -------
                    ps_dt = psG1.tile([128, L], f32, tag="ps_dt",
                                      name="ps_dt")
                    for j in range(2 * CT):
                        touch(xt[j][:])
                    with tc.tile_pool(name="stc", bufs=1) as cpool:
                        for d in range(2):
                            for f in range(2):
                                for k in range(CT):
                                    nc.tensor.matmul(
                                        ps_dt[0:XD, 512 * f:512 * (f + 1)],
                                        xpw_t[d * CT + k][:, 0:XD],
                                        xt[d * CT + k][:,
                                                       512 * f:512 * (f + 1)],
                                        start=(k == 0), stop=(k == CT - 1))
                            xd_s = cpool.tile([XD, L], f32, tag="xd_s",
                                              bufs=2, name=f"xd_s{d}")
                            nc.scalar.activation(xd_s[:], ps_dt[0:XD, :],
                                                 AF.Copy)
                            nc.sync.dma_start(ar_in[XD * d:XD * (d + 1), :],
                                              xd_s[:])

                    # ------- Stage D: AllReduce over d-shard groups -------
                    nc.gpsimd.collective_compute(
                        "AllReduce", OP.add,
                        replica_groups=[[0, 1, 2, 3], [4, 5, 6, 7]],
                        ins=[ar_in[:].opt()], outs=[ar_out[:].opt()])

                    # ------- Stage E: delta + du + y_acc init -------
                    delta = [mp.tile([128, L], f32, tag=f"dl{j}",
                                     name=f"dl{j}") for j in range(2 * CT)]
                    du = [mp.tile([128, L], f32, tag=f"du{j}", name=f"du{j}")
                          for j in range(2 * CT)]
                    y_acc = [mp.tile([128, L], f32, tag=f"ya{j}",
                                     name=f"ya{j}") for j in range(2 * CT)]
                    if True:
                        for d in range(2):
                            dtv = mp.tile([DT_RANK, L], f32, tag="dtv",
                                          bufs=1, name=f"dtv{d}")
                            dtv2 = mp.tile([DT_RANK, L], f32, tag="dtv2",
                                           bufs=1, name=f"dtv2{d}")
                            nc.sync.dma_start(
                                dtv[:], ar_out[XD * d:XD * d + DT_RANK, :])
                            # single-writer ACT copy so the matmuls wait on
                            # one semaphore (the DMA fans out across queues)
                            nc.scalar.activation(dtv2[:], dtv[:], AF.Copy)
                            for i in range(CT):
                                for f in range(2):
                                    nc.tensor.matmul(
                                        ps_dt[:, 512 * f:512 * (f + 1)],
                                        dtw_t[d][:, 128 * i:128 * (i + 1)],
                                        dtv2[:, 512 * f:512 * (f + 1)],
                                        start=True, stop=True)
                                c = cst[d * CT + i]
                                j = d * CT + i
                                # softplus(x + b) = ln(1 + exp(x + b))
                                nc.scalar.activation(delta[j][:], ps_dt[:],
                                                     AF.Exp, bias=c[:, 5:6])
                                nc.scalar.activation(delta[j][:],
                                                     delta[j][:], AF.Ln,
                                                     bias=1.0)
                                nc.vector.tensor_mul(du[j][:], delta[j][:],
                                                     xt[j][:])
                                # init y accumulator with the xt*D skip term
                                nc.vector.scalar_tensor_tensor(
                                    y_acc[j][:], xt[j][:], c[:, 6:7],
                                    xt[j][:], op0=OP.mult, op1=OP.bypass)

                # ------- Stage F: selective scan -------
                with tc.tile_pool(name="stf", bufs=1) as fp:
                    bb = psG1.tile([128, L], f32, tag="bb", name="bb")
                    cb = psG1.tile([128, L], f32, tag="cb", name="cb")
                    for d in range(2):
                        bcs = mp.tile([2 * D_STATE, L], f32, tag="bcs",
                                      bufs=1, name=f"bcs{d}")
                        bcf = mp.tile([2 * D_STATE, L], bf16, tag="bcf",
                                      bufs=1, name=f"bcf{d}")
                        nc.sync.dma_start(
                            bcs[:], ar_out[XD * d + DT_RANK:
                                           XD * d + DT_RANK + 2 * D_STATE, :])
                        nc.vector.tensor_copy(bcf[:], bcs[:])
                        for n in range(D_STATE):
                            for f in range(2):
                                fs = slice(512 * f, 512 * (f + 1))
                                nc.tensor.matmul(
                                    bb[:, fs],
                                    selt[:, 128 * n:128 * (n + 1)],
                                    bcf[:, fs], start=True, stop=True)
                                nc.tensor.matmul(
                                    cb[:, fs],
                                    selt[:, 128 * (D_STATE + n):
                                          128 * (D_STATE + n + 1)],
                                    bcf[:, fs], start=True, stop=True)
                            for i in range(CT):
                                j = d * CT + i
                                c = cst[j]
                                a_t = fp.tile([128, L], f32, tag="a_t",
                                              bufs=4, name=f"a{d}_{n}_{i}")
                                b_t = fp.tile([128, L], f32, tag="b_t",
                                              bufs=4, name=f"b{d}_{n}_{i}")
                                h_t = fp.tile([128, L], f32, tag="h_t",
                                              bufs=4, name=f"h{d}_{n}_{i}")
                                nc.vector.tensor_mul(b_t[:], du[j][:], bb[:])
                                nc.scalar.activation(
                                    a_t[:], delta[j][:], AF.Exp,
                                    scale=c[:, 7 + n:8 + n])
                                nc.gpsimd.memset(a_t[:, 0:1], 0.0)
                                nc.vector.tensor_tensor_scan(
                                    h_t[:], a_t[:], b_t[:], 0.0,
                                    op0=OP.mult, op1=OP.add)
                                nc.vector.tensor_mul(h_t[:], h_t[:], cb[:])
                                nc.vector.tensor_add(y_acc[j][:],
                                                     y_acc[j][:], h_t[:])

                # ------- Stage G/H: combine directions + gate -------
                for i in range(CT):
                    nc.vector.tensor_add(y_acc[i][:], y_acc[i][:],
                                         rev_view(y_acc[CT + i][:]))
                    nc.vector.tensor_mul(y_acc[i][:], y_acc[i][:], sz[i][:])
                    touch(y_acc[i][:])

            # ---------- Stage I: out_proj partial ----------
            with tc.tile_pool(name="sti", bufs=1) as ip:
                for o in range(6):
                    for f in range(2):
                        for k in range(CT):
                            nc.tensor.matmul(
                                ps_dt[:, 512 * f:512 * (f + 1)],
                                wo_t[k][:, 128 * o:128 * (o + 1)],
                                y_acc[k][:, 512 * f:512 * (f + 1)],
                                start=(k == 0), stop=(k == CT - 1))
                    o_s = ip.tile([128, L], f32, tag="o_s", bufs=2,
                                  name=f"o_s{o}")
                    nc.scalar.activation(o_s[:], ps_dt[:], AF.Copy)
                    nc.sync.dma_start(outp[128 * o:128 * (o + 1), :], o_s[:])

    nc.compile()
    return nc


def _prep_inputs(inputs):
    import ml_dtypes

    h = np.asarray(inputs["hidden_states"], np.float32)
    w_in_full = np.asarray(inputs["in_proj_w"], np.float32)
    wo_full = np.asarray(inputs["out_proj_w"], np.float32)
    sel = np.zeros((2 * D_STATE, 2 * D_STATE * 128), np.float32)
    for n in range(2 * D_STATE):
        sel[n, 128 * n:128 * (n + 1)] = 1.0
    sel = sel.astype(ml_dtypes.bfloat16)

    pd = {}
    for d, tag in enumerate(["f", "r"]):
        pd[d] = dict(
            conv_w=np.asarray(inputs[f"conv_w_{tag}"], np.float32)[:, 0, :],
            conv_b=np.asarray(inputs[f"conv_b_{tag}"], np.float32),
            xp=np.asarray(inputs[f"x_proj_w_{tag}"], np.float32),
            dtw=np.asarray(inputs[f"dt_w_{tag}"], np.float32),
            dtb=np.asarray(inputs[f"dt_b_{tag}"], np.float32),
            A=-np.exp(np.asarray(inputs[f"A_log_{tag}"], np.float32)),
            D=np.asarray(inputs[f"D_{tag}"], np.float32),
        )

    in_maps = []
    for c in range(N_CORES):
        b, g = c // N_GROUPS, c % N_GROUPS
        sl = slice(CH * g, CH * (g + 1))
        m = {}
        m["hT"] = np.ascontiguousarray(h[b].T)
        rows = np.r_[CH * g:CH * (g + 1),
                     D_INNER + CH * g:D_INNER + CH * (g + 1)]
        m["w_in"] = np.ascontiguousarray(w_in_full[rows, :].T)
        cstm = np.zeros((2 * CH, 23), np.float32)
        xp_wm = np.zeros((2 * CH, 80), np.float32)
        dt_wm = np.zeros((2 * DT_RANK, CH), np.float32)
        for d in range(2):
            p = pd[d]
            r = slice(CH * d, CH * (d + 1))
            cstm[r, 0:4] = p["conv_w"][sl]
            cstm[r, 4] = p["conv_b"][sl]
            cstm[r, 5] = p["dtb"][sl]
            cstm[r, 6] = p["D"][sl]
            cstm[r, 7:23] = p["A"][sl]
            xp_wm[r, :] = p["xp"][:, sl].T
            dt_wm[DT_RANK * d:DT_RANK * (d + 1), :] = p["dtw"][sl, :].T
        m["consts"] = cstm
        m["xp_w"] = xp_wm
        m["dt_wT"] = dt_wm
        m["wo"] = np.ascontiguousarray(wo_full[:, sl].T)
        m["sel"] = sel
        in_maps.append(m)
    return in_maps


def kernel(**inputs):
    from concourse import bass_utils

    if "nc" not in _cache:
        _cache["nc"] = _build()
    nc = _cache["nc"]
    in_maps = _prep_inputs(inputs)
    res = bass_utils.run_bass_kernel_spmd(nc, in_maps,
                                          core_ids=list(range(N_CORES)))
    outs = [r["outp"] for r in res.results]
    full = np.empty((B, L, D_MODEL), np.float32)
    for b in range(B):
        acc = outs[b * N_GROUPS].astype(np.float64)
        for g in range(1, N_GROUPS):
            acc = acc + outs[b * N_GROUPS + g]
        full[b] = acc.T
    return full


# revision 36
# speedup vs baseline: 3727.5785x; 3727.5785x over previous
"""BiMamba (bidirectional Mamba block) Trainium2 kernel.

Sharding: 8 cores = 2 (batch) x 4 (d_inner quarters of 384 channels).
Per core: in_proj (x,z slices) -> depthwise causal conv (fwd + reversed)
-> silu -> x_proj partials -> AllReduce over the 4-core d-shard group
-> delta(softplus) -> selective scan via DVE tensor_tensor_scan (channels
on partitions, state segments along the free axis) -> gating -> out_proj
partial.  Host sums the 4 partials per batch.

flip(L) commutes with L-wise matmuls, so in_proj/out_proj run once:
  out = (Y_f + flip(Y_r)) * silu(z) @ W_o^T.

This walrus build supports only ONE sync-wait per Matmult, so the kernel is
structured so every matmul carries at most one wait:
  - all PE-feeding weight tiles are DMA'd and "touched" (1x1 self-matmul
    into a scratch PSUM corner) in a preamble, putting their DMA semaphores
    into the PE's observed vector clock;
  - PSUM tiles are allocated once per tag and overwritten (no slot
    recycling), so a group's first matmul waits only on the single-engine
    WAR release of the previous group's readers;
  - stages A..E share one PSUM pool (no pool transition deps); the F and I
    pools' transition deps are absorbed by DVE memset guards, which
    coalesce with the (also DVE) data dependency of their first matmuls.
"""

import numpy as np

D_MODEL = 768
D_INNER = 1536
D_STATE = 16
D_CONV = 4
DT_RANK = 48
L = 1024
B = 2
N_CORES = 8
N_GROUPS = 4               # d-shard groups
CH = D_INNER // N_GROUPS   # 384 channels per core
CT = CH // 128             # 3 channel partition-tiles
NSEG = 2                   # state segments per scan chunk
NCHUNK = D_STATE // NSEG
XD = DT_RANK + 2 * D_STATE  # 80

_cache = {}


def _build(single_core=False):
    import concourse.bass as bass
    import concourse.bacc as bacc
    import concourse.mybir as mybir
    from concourse import tile

    f32 = mybir.dt.float32
    bf16 = mybir.dt.bfloat16
    AF = mybir.ActivationFunctionType
    OP = mybir.AluOpType

    nc = bacc.Bacc("TRN2", target_bir_lowering=False, debug=False,
                   num_devices=1 if single_core else N_CORES)

    hT = nc.dram_tensor("hT", [D_MODEL, L], f32, kind="ExternalInput").ap()
    w_in = nc.dram_tensor("w_in", [D_MODEL, 2 * CH], f32, kind="ExternalInput").ap()
    consts = nc.dram_tensor("consts", [2 * CH, 23], f32, kind="ExternalInput").ap()
    xp_w = nc.dram_tensor("xp_w", [2 * CH, 80], f32, kind="ExternalInput").ap()
    dt_wT = nc.dram_tensor("dt_wT", [2 * DT_RANK, CH], f32, kind="ExternalInput").ap()
    wo = nc.dram_tensor("wo", [CH, D_MODEL], f32, kind="ExternalInput").ap()
    sel_d = nc.dram_tensor("sel", [2 * D_STATE, 2 * D_STATE * 128], bf16,
                           kind="ExternalInput").ap()
    outp = nc.dram_tensor("outp", [D_MODEL, L], f32, kind="ExternalOutput").ap()

    def rev_view(ap, n=L):
        return bass.AP(ap.tensor, ap.offset + (n - 1), [ap.ap[0], [-1, n]])

    def bcast_seg(ap, nseg):
        return bass.AP(ap.tensor, ap.offset, [ap.ap[0], [0, nseg], [1, L]])

    with tile.TileContext(nc) as tc:
        with (
            tc.tile_pool(name="persist", bufs=1) as pp,
            tc.tile_pool(name="dram", bufs=1, space="DRAM") as dp,
            tc.tile_pool(name="mid", bufs=1) as mp,
            tc.tile_pool(name="psG1", bufs=1, space="PSUM") as psG1,
        ):
            # ---------- persistent tiles + PE preamble touchers ----------
            scr = psG1.tile([1, 8], f32, tag="scr", name="scr")

            def touch(ap):
                nc.tensor.matmul(scr[0:1, 0:1], ap[:, 0:1], ap[:, 0:1],
                                 start=True, stop=True)

            sz = [pp.tile([128, L], f32, tag=f"sz{i}", name=f"sz{i}")
                  for i in range(CT)]
            cst = [pp.tile([128, 23], f32, tag=f"cst{j}", name=f"cst{j}")
                   for j in range(2 * CT)]
            for j in range(2 * CT):
                nc.sync.dma_start(cst[j][:], consts[128 * j:128 * (j + 1), :])
            touch(cst[0][:])
            selt = pp.tile([2 * D_STATE, 2 * D_STATE * 128], bf16, tag="sel",
                           name="selt")
            nc.sync.dma_start(selt[:], sel_d[:])
            touch(selt[:])
            xpw_t = []
            for j in range(2 * CT):
                w_j = pp.tile([128, 80], f32, tag=f"xpw{j}", name=f"xpw{j}")
                nc.sync.dma_start(w_j[:], xp_w[128 * j:128 * (j + 1), :])
                touch(w_j[:])
                xpw_t.append(w_j)
            dtw_t = []
            for d in range(2):
                w_d = pp.tile([DT_RANK, CH], f32, tag=f"dtw{d}",
                              name=f"dtw{d}")
                nc.sync.dma_start(w_d[:],
                                  dt_wT[DT_RANK * d:DT_RANK * (d + 1), :])
                touch(w_d[:])
                dtw_t.append(w_d)
            wo_t = []
            for k in range(CT):
                w_k = pp.tile([128, D_MODEL], f32, tag=f"wo{k}",
                              name=f"wo{k}")
                nc.sync.dma_start(w_k[:], wo[128 * k:128 * (k + 1), :])
                touch(w_k[:])
                wo_t.append(w_k)

            ar_in = dp.tile([2 * XD, L], f32, name="ar_in")
            ar_out = dp.tile([2 * XD, L], f32, name="ar_out")

            if True:
                with tc.tile_pool(name="xtp", bufs=1) as xp_pool:
                    xt = [xp_pool.tile([128, L], f32, tag=f"xt{j}",
                                       name=f"xt{j}") for j in range(2 * CT)]

                    # ------- Stage A: in_proj; Stage B: conv + silu -------
                    with tc.tile_pool(name="stab", bufs=1) as ab:
                        ps_in = psG1.tile([128, 512], f32, tag="ps_in",
                                          name="ps_in")
                        xpad = [ab.tile([128, L + D_CONV - 1], f32,
                                        tag=f"xp{i}", name=f"xp{i}")
                                for i in range(CT)]
                        xrpad = [ab.tile([128, L + D_CONV - 1], f32,
                                         tag=f"xr{i}", name=f"xr{i}")
                                 for i in range(CT)]
                        for i in range(CT):
                            nc.gpsimd.memset(xpad[i][:, 0:D_CONV - 1], 0.0)
                            nc.gpsimd.memset(xrpad[i][:, 0:D_CONV - 1], 0.0)
                        for f in range(2):
                            hTc = []
                            for k in range(6):
                                h_k = ab.tile([128, 512], f32, tag=f"hTc{k}",
                                              name=f"hTc{f}_{k}")
                                nc.sync.dma_start(
                                    h_k[:], hT[128 * k:128 * (k + 1),
                                               512 * f:512 * (f + 1)])
                                touch(h_k[:])
                                hTc.append(h_k)
                            for o in range(6):
                                win_t = []
                                for k in range(6):
                                    w_k = ab.tile(
                                        [128, 128], f32, tag=f"win{k}",
                                        bufs=2, name=f"win{f}_{o}_{k}")
                                    nc.sync.dma_start(
                                        w_k[:], w_in[128 * k:128 * (k + 1),
                                                     128 * o:128 * (o + 1)])
                                    touch(w_k[:])
                                    win_t.append(w_k)
                                ps = ps_in
                                for k in range(6):
                                    nc.tensor.matmul(ps[:], win_t[k][:],
                                                     hTc[k][:],
                                                     start=(k == 0),
                                                     stop=(k == 5))
                                if o < CT:
                                    pa = D_CONV - 1
                                    dst = xpad[o][:, pa + 512 * f:
                                                  pa + 512 * (f + 1)]
                                    nc.scalar.activation(dst, ps[:], AF.Copy)
                                    nc.vector.tensor_copy(
                                        xrpad[o][:, pa + 512 * (1 - f):
                                                 pa + 512 * (2 - f)],
                                        rev_view(dst, 512))
                                else:
                                    i = o - CT
                                    sl = slice(512 * f, 512 * (f + 1))
                                    zc = ab.tile([128, 512], f32,
                                                 tag="cacc",
                                                 bufs=2, name=f"zc{f}_{o}")
                                    nc.scalar.activation(zc[:], ps[:],
                                                         AF.Copy)
                                    nc.scalar.activation(sz[i][:, sl], ps[:],
                                                         AF.Sigmoid)
                                    nc.vector.tensor_mul(sz[i][:, sl],
                                                         sz[i][:, sl], zc[:])

                        for d in range(2):
                            src = xpad if d == 0 else xrpad
                            for i in range(CT):
                                c = cst[d * CT + i]
                                for hh in range(2):
                                    c0 = 512 * hh
                                    acc = ab.tile([128, 512], f32,
                                                  tag="cacc", bufs=2,
                                                  name=f"cacc{d}_{i}_{hh}")
                                    nc.vector.scalar_tensor_tensor(
                                        acc[:], src[i][:, c0:c0 + 512],
                                        c[:, 0:1], src[i][:, c0:c0 + 512],
                                        op0=OP.mult, op1=OP.bypass)
                                    for k in range(1, D_CONV):
                                        nc.vector.scalar_tensor_tensor(
                                            acc[:],
                                            src[i][:, c0 + k:c0 + k + 512],
                                            c[:, k:k + 1], acc[:],
                                            op0=OP.mult, op1=OP.add)
                                    xb = ab.tile([128, 512], f32, tag="xb",
                                                 bufs=2,
                                                 name=f"xb{d}_{i}_{hh}")
                                    sg = ab.tile([128, 512], f32, tag="sg",
                                                 bufs=2,
                                                 name=f"sg{d}_{i}_{hh}")
                                    nc.scalar.activation(xb[:], acc[:],
                                                         AF.Identity,
                                                         bias=c[:, 4:5])
                                    nc.scalar.activation(sg[:], acc[:],
                                                         AF.Sigmoid,
                                                         bias=c[:, 4:5])
                                    nc.vector.tensor_mul(
                                        xt[d * CT + i][:, c0:c0 + 512],
                                        sg[:], xb[:])

                    # ------- Stage C: x_proj partials -------
                    ps_dt = psG1.tile([128, L], f32, tag="ps_dt",
                                      name="ps_dt")
                    for j in range(2 * CT):
                        touch(xt[j][:])
                    with tc.tile_pool(name="stc", bufs=1) as cpool:
                        for d in range(2):
                            for f in range(2):
                                for k in range(CT):
                                    nc.tensor.matmul(
                                        ps_dt[0:XD, 512 * f:512 * (f + 1)],
                                        xpw_t[d * CT + k][:, 0:XD],
                                        xt[d * CT + k][:,
                                                       512 * f:512 * (f + 1)],
                                        start=(k == 0), stop=(k == CT - 1))
                            xd_s = cpool.tile([XD, L], f32, tag="xd_s",
                                              bufs=2, name=f"xd_s{d}")
                            nc.scalar.activation(xd_s[:], ps_dt[0:XD, :],
                                                 AF.Copy)
                            nc.sync.dma_start(ar_in[XD * d:XD * (d + 1), :],
                                              xd_s[:])

                    # ------- Stage D: AllReduce over d-shard groups -------
                    if single_core:
                        nc.sync.dma_start(ar_out[:], ar_in[:])
                    else:
                        nc.gpsimd.collective_compute(
                            "AllReduce", OP.add,
                            replica_groups=[[0, 1, 2, 3], [4, 5, 6, 7]],
                            ins=[ar_in[:].opt()], outs=[ar_out[:].opt()])

                    # ------- Stage E: delta + du + y_acc init -------
                    delta = [mp.tile([128, L], f32, tag=f"dl{j}",
                                     name=f"dl{j}") for j in range(2 * CT)]
                    du = [mp.tile([128, L], f32, tag=f"du{j}", name=f"du{j}")
                          for j in range(2 * CT)]
                    y_acc = [mp.tile([128, L], f32, tag=f"ya{j}",
                                     name=f"ya{j}") for j in range(2 * CT)]
                    if True:
                        for d in range(2):
                            dtv = mp.tile([DT_RANK, L], f32, tag="dtv",
                                          bufs=1, name=f"dtv{d}")
                            dtv2 = mp.tile([DT_RANK, L], f32, tag="dtv2",
                                           bufs=1, name=f"dtv2{d}")
                            nc.sync.dma_start(
                                dtv[:], ar_out[XD * d:XD * d + DT_RANK, :])
                            # single-writer ACT copy so the matmuls wait on
                            # one semaphore (the DMA fans out across queues)
                            nc.scalar.activation(dtv2[:], dtv[:], AF.Copy)
                            for i in range(CT):
                                for f in range(2):
                                    nc.tensor.matmul(
                                        ps_dt[:, 512 * f:512 * (f + 1)],
                                        dtw_t[d][:, 128 * i:128 * (i + 1)],
                                        dtv2[:, 512 * f:512 * (f + 1)],
                                        start=True, stop=True)
                                c = cst[d * CT + i]
                                j = d * CT + i
                                # softplus(x + b) = ln(1 + exp(x + b))
                                nc.scalar.activation(delta[j][:], ps_dt[:],
                                                     AF.Exp, bias=c[:, 5:6])
                                nc.scalar.activation(delta[j][:],
                                                     delta[j][:], AF.Ln,
                                                     bias=1.0)
                                nc.vector.tensor_mul(du[j][:], delta[j][:],
                                                     xt[j][:])
                                # init y accumulator with the xt*D skip term
                                nc.vector.scalar_tensor_tensor(
                                    y_acc[j][:], xt[j][:], c[:, 6:7],
                                    xt[j][:], op0=OP.mult, op1=OP.bypass)

                # ------- Stage F: selective scan -------
                with tc.tile_pool(name="stf", bufs=1) as fp:
                    bb = psG1.tile([128, L], f32, tag="bb", name="bb")
                    cb = psG1.tile([128, L], f32, tag="cb", name="cb")
                    for d in range(2):
                        bcs = mp.tile([2 * D_STATE, L], f32, tag="bcs",
                                      bufs=1, name=f"bcs{d}")
                        bcf = mp.tile([2 * D_STATE, L], bf16, tag="bcf",
                                      bufs=1, name=f"bcf{d}")
                        nc.sync.dma_start(
                            bcs[:], ar_out[XD * d + DT_RANK:
                                           XD * d + DT_RANK + 2 * D_STATE, :])
                        nc.vector.tensor_copy(bcf[:], bcs[:])
                        for n in range(D_STATE):
                            for f in range(2):
                                fs = slice(512 * f, 512 * (f + 1))
                                nc.tensor.matmul(
                                    bb[:, fs],
                                    selt[:, 128 * n:128 * (n + 1)],
                                    bcf[:, fs], start=True, stop=True)
                                nc.tensor.matmul(
                                    cb[:, fs],
                                    selt[:, 128 * (D_STATE + n):
                                          128 * (D_STATE + n + 1)],
                                    bcf[:, fs], start=True, stop=True)
                            for i in range(CT):
                                j = d * CT + i
                                c = cst[j]
                                a_t = fp.tile([128, L], f32, tag="a_t",
                                              bufs=4, name=f"a{d}_{n}_{i}")
                                b_t = fp.tile([128, L], f32, tag="b_t",
                                              bufs=4, name=f"b{d}_{n}_{i}")
                                h_t = fp.tile([128, L], f32, tag="h_t",
                                              bufs=4, name=f"h{d}_{n}_{i}")
                                nc.vector.tensor_mul(b_t[:], du[j][:], bb[:])
                                nc.scalar.activation(
                                    a_t[:], delta[j][:], AF.Exp,
                                    scale=c[:, 7 + n:8 + n])
                                nc.gpsimd.memset(a_t[:, 0:1], 0.0)
                                nc.vector.tensor_tensor_scan(
                                    h_t[:], a_t[:], b_t[:], 0.0,
                                    op0=OP.mult, op1=OP.add)
                                nc.vector.tensor_mul(h_t[:], h_t[:], cb[:])
                                nc.vector.tensor_add(y_acc[j][:],
                                                     y_acc[j][:], h_t[:])

                # ------- Stage G/H: combine directions + gate -------
                for i in range(CT):
                    nc.vector.tensor_add(y_acc[i][:], y_acc[i][:],
                                         rev_view(y_acc[CT + i][:]))
                    nc.vector.tensor_mul(y_acc[i][:], y_acc[i][:], sz[i][:])
                    touch(y_acc[i][:])

            # ---------- Stage I: out_proj partial ----------
            with tc.tile_pool(name="sti", bufs=1) as ip:
                for o in range(6):
                    for f in range(2):
                        for k in range(CT):
                            nc.tensor.matmul(
                                ps_dt[:, 512 * f:512 * (f + 1)],
                                wo_t[k][:, 128 * o:128 * (o + 1)],
                                y_acc[k][:, 512 * f:512 * (f + 1)],
                                start=(k == 0), stop=(k == CT - 1))
                    o_s = ip.tile([128, L], f32, tag="o_s", bufs=2,
                                  name=f"o_s{o}")
                    nc.scalar.activation(o_s[:], ps_dt[:], AF.Copy)
                    nc.sync.dma_start(outp[128 * o:128 * (o + 1), :], o_s[:])

    nc.compile()
    return nc


def _prep_inputs(inputs):
    import ml_dtypes

    h = np.asarray(inputs["hidden_states"], np.float32)
    w_in_full = np.asarray(inputs["in_proj_w"], np.float32)
    wo_full = np.asarray(inputs["out_proj_w"], np.float32)
    sel = np.zeros((2 * D_STATE, 2 * D_STATE * 128), np.float32)
    for n in range(2 * D_STATE):
        sel[n, 128 * n:128 * (n + 1)] = 1.0
    sel = sel.astype(ml_dtypes.bfloat16)

    pd = {}
    for d, tag in enumerate(["f", "r"]):
        pd[d] = dict(
            conv_w=np.asarray(inputs[f"conv_w_{tag}"], np.float32)[:, 0, :],
            conv_b=np.asarray(inputs[f"conv_b_{tag}"], np.float32),
            xp=np.asarray(inputs[f"x_proj_w_{tag}"], np.float32),
            dtw=np.asarray(inputs[f"dt_w_{tag}"], np.float32),
            dtb=np.asarray(inputs[f"dt_b_{tag}"], np.float32),
            A=-np.exp(np.asarray(inputs[f"A_log_{tag}"], np.float32)),
            D=np.asarray(inputs[f"D_{tag}"], np.float32),
        )

    in_maps = []
    for c in range(N_CORES):
        b, g = c // N_GROUPS, c % N_GROUPS
        sl = slice(CH * g, CH * (g + 1))
        m = {}
        m["hT"] = np.ascontiguousarray(h[b].T)
        rows = np.r_[CH * g:CH * (g + 1),
                     D_INNER + CH * g:D_INNER + CH * (g + 1)]
        m["w_in"] = np.ascontiguousarray(w_in_full[rows, :].T)
        cstm = np.zeros((2 * CH, 23), np.float32)
        xp_wm = np.zeros((2 * CH, 80), np.float32)
        dt_wm = np.zeros((2 * DT_RANK, CH), np.float32)
        for d in range(2):
            p = pd[d]
            r = slice(CH * d, CH * (d + 1))
            cstm[r, 0:4] = p["conv_w"][sl]
            cstm[r, 4] = p["conv_b"][sl]
            cstm[r, 5] = p["dtb"][sl]
            cstm[r, 6] = p["D"][sl]
            cstm[r, 7:23] = p["A"][sl]
            xp_wm[r, :] = p["xp"][:, sl].T
            dt_wm[DT_RANK * d:DT_RANK * (d + 1), :] = p["dtw"][sl, :].T
        m["consts"] = cstm
        m["xp_w"] = xp_wm
        m["dt_wT"] = dt_wm
        m["wo"] = np.ascontiguousarray(wo_full[:, sl].T)
        m["sel"] = sel
        in_maps.append(m)
    return in_maps


def kernel(**inputs):
    from concourse import bass_utils

    if "nc" not in _cache:
        _cache["nc"] = _build()
    nc = _cache["nc"]
    in_maps = _prep_inputs(inputs)
    res = bass_utils.run_bass_kernel_spmd(nc, in_maps,
                                          core_ids=list(range(N_CORES)))
    outs = [r["outp"] for r in res.results]
    full = np.empty((B, L, D_MODEL), np.float32)
    for b in range(B):
        acc = outs[b * N_GROUPS].astype(np.float64)
        for g in range(1, N_GROUPS):
            acc = acc + outs[b * N_GROUPS + g]
        full[b] = acc.T
    return full


# revision 42
# speedup vs baseline: 4618.2165x; 1.2389x over previous
"""BiMamba (bidirectional Mamba block) Trainium2 kernel.

Sharding: 8 cores = 2 (batch) x 4 (d_inner quarters of 384 channels).
Per core: in_proj (x,z slices) -> depthwise causal conv (fwd + reversed)
-> silu -> x_proj partials -> AllReduce over the 4-core d-shard group
-> delta(softplus) -> selective scan via DVE tensor_tensor_scan (channels
on partitions, state segments along the free axis) -> gating -> out_proj
partial.  Host sums the 4 partials per batch.

flip(L) commutes with L-wise matmuls, so in_proj/out_proj run once:
  out = (Y_f + flip(Y_r)) * silu(z) @ W_o^T.

This walrus build supports only ONE sync-wait per Matmult, so the kernel is
structured so every matmul carries at most one wait:
  - all PE-feeding weight tiles are DMA'd and "touched" (1x1 self-matmul
    into a scratch PSUM corner) in a preamble, putting their DMA semaphores
    into the PE's observed vector clock;
  - PSUM tiles are allocated once per tag and overwritten (no slot
    recycling), so a group's first matmul waits only on the single-engine
    WAR release of the previous group's readers;
  - stages A..E share one PSUM pool (no pool transition deps); the F and I
    pools' transition deps are absorbed by DVE memset guards, which
    coalesce with the (also DVE) data dependency of their first matmuls.
"""

import numpy as np

D_MODEL = 768
D_INNER = 1536
D_STATE = 16
D_CONV = 4
DT_RANK = 48
L = 1024
B = 2
N_CORES = 8
N_GROUPS = 4               # d-shard groups
CH = D_INNER // N_GROUPS   # 384 channels per core
CT = CH // 128             # 3 channel partition-tiles
NSEG = 2                   # state segments per scan chunk
NCHUNK = D_STATE // NSEG
XD = DT_RANK + 2 * D_STATE  # 80

_cache = {}


def _build(single_core=False):
    import concourse.bass as bass
    import concourse.bacc as bacc
    import concourse.mybir as mybir
    from concourse import tile

    f32 = mybir.dt.float32
    bf16 = mybir.dt.bfloat16
    AF = mybir.ActivationFunctionType
    OP = mybir.AluOpType

    nc = bacc.Bacc("TRN2", target_bir_lowering=False, debug=False,
                   num_devices=1 if single_core else N_CORES)

    hT = nc.dram_tensor("hT", [D_MODEL, L], f32, kind="ExternalInput").ap()
    w_in = nc.dram_tensor("w_in", [D_MODEL, 2 * CH], f32, kind="ExternalInput").ap()
    consts = nc.dram_tensor("consts", [2 * CH, 23], f32, kind="ExternalInput").ap()
    xp_w = nc.dram_tensor("xp_w", [2 * CH, 80], f32, kind="ExternalInput").ap()
    dt_wT = nc.dram_tensor("dt_wT", [2 * DT_RANK, CH], f32, kind="ExternalInput").ap()
    wo = nc.dram_tensor("wo", [CH, D_MODEL], f32, kind="ExternalInput").ap()
    sel_d = nc.dram_tensor("sel", [2 * D_STATE, 2 * D_STATE * 128], bf16,
                           kind="ExternalInput").ap()
    outp = nc.dram_tensor("outp", [D_MODEL, L], f32, kind="ExternalOutput").ap()

    def rev_view(ap, n=L):
        return bass.AP(ap.tensor, ap.offset + (n - 1), [ap.ap[0], [-1, n]])

    def bcast_seg(ap, nseg):
        return bass.AP(ap.tensor, ap.offset, [ap.ap[0], [0, nseg], [1, L]])

    with tile.TileContext(nc) as tc:
        with (
            tc.tile_pool(name="persist", bufs=1) as pp,
            tc.tile_pool(name="dram", bufs=1, space="DRAM") as dp,
            tc.tile_pool(name="mid", bufs=1) as mp,
            tc.tile_pool(name="psG1", bufs=1, space="PSUM") as psG1,
        ):
            # ---------- persistent tiles + PE preamble touchers ----------
            scr = psG1.tile([1, 8], f32, tag="scr", name="scr")

            def touch(ap):
                nc.tensor.matmul(scr[0:1, 0:1], ap[:, 0:1], ap[:, 0:1],
                                 start=True, stop=True)

            sz = [pp.tile([128, L], f32, tag=f"sz{i}", name=f"sz{i}")
                  for i in range(CT)]
            cst = [pp.tile([128, 23], f32, tag=f"cst{j}", name=f"cst{j}")
                   for j in range(2 * CT)]
            for j in range(2 * CT):
                nc.sync.dma_start(cst[j][:], consts[128 * j:128 * (j + 1), :])
            touch(cst[0][:])
            selt = pp.tile([2 * D_STATE, 2 * D_STATE * 128], bf16, tag="sel",
                           name="selt")
            nc.sync.dma_start(selt[:], sel_d[:])
            touch(selt[:])
            xpw_t = []
            for j in range(2 * CT):
                w_j = pp.tile([128, 80], f32, tag=f"xpw{j}", name=f"xpw{j}")
                nc.sync.dma_start(w_j[:], xp_w[128 * j:128 * (j + 1), :])
                touch(w_j[:])
                xpw_t.append(w_j)
            dtw_t = []
            for d in range(2):
                w_d = pp.tile([DT_RANK, CH], f32, tag=f"dtw{d}",
                              name=f"dtw{d}")
                nc.sync.dma_start(w_d[:],
                                  dt_wT[DT_RANK * d:DT_RANK * (d + 1), :])
                touch(w_d[:])
                dtw_t.append(w_d)
            wo_t = []
            for k in range(CT):
                w_k = pp.tile([128, D_MODEL], f32, tag=f"wo{k}",
                              name=f"wo{k}")
                nc.sync.dma_start(w_k[:], wo[128 * k:128 * (k + 1), :])
                touch(w_k[:])
                wo_t.append(w_k)

            ar_in = dp.tile([2 * XD, L], f32, name="ar_in")
            ar_out = dp.tile([2 * XD, L], f32, name="ar_out")

            if True:
                with tc.tile_pool(name="xtp", bufs=1) as xp_pool:
                    xt = [xp_pool.tile([128, L], f32, tag=f"xt{j}",
                                       name=f"xt{j}") for j in range(2 * CT)]

                    # ------- Stage A: in_proj; Stage B: conv + silu -------
                    with tc.tile_pool(name="stab", bufs=1) as ab:
                        ps_in = psG1.tile([128, 512], f32, tag="ps_in",
                                          name="ps_in")
                        xpad = [ab.tile([128, L + D_CONV - 1], f32,
                                        tag=f"xp{i}", name=f"xp{i}")
                                for i in range(CT)]
                        xrpad = [ab.tile([128, L + D_CONV - 1], f32,
                                         tag=f"xr{i}", name=f"xr{i}")
                                 for i in range(CT)]
                        for i in range(CT):
                            nc.gpsimd.memset(xpad[i][:, 0:D_CONV - 1], 0.0)
                            nc.gpsimd.memset(xrpad[i][:, 0:D_CONV - 1], 0.0)
                        for f in range(2):
                            hTc = []
                            for k in range(6):
                                h_k = ab.tile([128, 512], f32, tag=f"hTc{k}",
                                              name=f"hTc{f}_{k}")
                                nc.sync.dma_start(
                                    h_k[:], hT[128 * k:128 * (k + 1),
                                               512 * f:512 * (f + 1)])
                                touch(h_k[:])
                                hTc.append(h_k)
                            for o in range(6):
                                win_t = []
                                for k in range(6):
                                    w_k = ab.tile(
                                        [128, 128], f32, tag=f"win{k}",
                                        bufs=2, name=f"win{f}_{o}_{k}")
                                    nc.sync.dma_start(
                                        w_k[:], w_in[128 * k:128 * (k + 1),
                                                     128 * o:128 * (o + 1)])
                                    touch(w_k[:])
                                    win_t.append(w_k)
                                ps = ps_in
                                for k in range(6):
                                    nc.tensor.matmul(ps[:], win_t[k][:],
                                                     hTc[k][:],
                                                     start=(k == 0),
                                                     stop=(k == 5))
                                if o < CT:
                                    pa = D_CONV - 1
                                    dst = xpad[o][:, pa + 512 * f:
                                                  pa + 512 * (f + 1)]
                                    nc.scalar.activation(dst, ps[:], AF.Copy)
                                    nc.vector.tensor_copy(
                                        xrpad[o][:, pa + 512 * (1 - f):
                                                 pa + 512 * (2 - f)],
                                        rev_view(dst, 512))
                                else:
                                    i = o - CT
                                    sl = slice(512 * f, 512 * (f + 1))
                                    zc = ab.tile([128, 512], f32,
                                                 tag="cacc",
                                                 bufs=2, name=f"zc{f}_{o}")
                                    nc.scalar.activation(zc[:], ps[:],
                                                         AF.Copy)
                                    nc.scalar.activation(sz[i][:, sl], ps[:],
                                                         AF.Sigmoid)
                                    nc.vector.tensor_mul(sz[i][:, sl],
                                                         sz[i][:, sl], zc[:])

                        for d in range(2):
                            src = xpad if d == 0 else xrpad
                            for i in range(CT):
                                c = cst[d * CT + i]
                                for hh in range(2):
                                    c0 = 512 * hh
                                    acc = ab.tile([128, 512], f32,
                                                  tag="cacc", bufs=2,
                                                  name=f"cacc{d}_{i}_{hh}")
                                    nc.vector.scalar_tensor_tensor(
                                        acc[:], src[i][:, c0:c0 + 512],
                                        c[:, 0:1], src[i][:, c0:c0 + 512],
                                        op0=OP.mult, op1=OP.bypass)
                                    for k in range(1, D_CONV):
                                        nc.vector.scalar_tensor_tensor(
                                            acc[:],
                                            src[i][:, c0 + k:c0 + k + 512],
                                            c[:, k:k + 1], acc[:],
                                            op0=OP.mult, op1=OP.add)
                                    xb = ab.tile([128, 512], f32, tag="xb",
                                                 bufs=2,
                                                 name=f"xb{d}_{i}_{hh}")
                                    sg = ab.tile([128, 512], f32, tag="sg",
                                                 bufs=2,
                                                 name=f"sg{d}_{i}_{hh}")
                                    nc.scalar.activation(xb[:], acc[:],
                                                         AF.Identity,
                                                         bias=c[:, 4:5])
                                    nc.scalar.activation(sg[:], acc[:],
                                                         AF.Sigmoid,
                                                         bias=c[:, 4:5])
                                    nc.vector.tensor_mul(
                                        xt[d * CT + i][:, c0:c0 + 512],
                                        sg[:], xb[:])

                    # ------- Stage C: x_proj partials -------
                    ps_dt = psG1.tile([128, L], f32, tag="ps_dt",
                                      name="ps_dt")
                    for j in range(2 * CT):
                        touch(xt[j][:])
                    with tc.tile_pool(name="stc", bufs=1) as cpool:
                        for d in range(2):
                            for f in range(2):
                                for k in range(CT):
                                    nc.tensor.matmul(
                                        ps_dt[0:XD, 512 * f:512 * (f + 1)],
                                        xpw_t[d * CT + k][:, 0:XD],
                                        xt[d * CT + k][:,
                                                       512 * f:512 * (f + 1)],
                                        start=(k == 0), stop=(k == CT - 1))
                            xd_s = cpool.tile([XD, L], f32, tag="xd_s",
                                              bufs=2, name=f"xd_s{d}")
                            nc.scalar.activation(xd_s[:], ps_dt[0:XD, :],
                                                 AF.Copy)
                            nc.sync.dma_start(ar_in[XD * d:XD * (d + 1), :],
                                              xd_s[:])

                    # ------- Stage D: AllReduce over d-shard groups -------
                    if single_core:
                        nc.sync.dma_start(ar_out[:], ar_in[:])
                    else:
                        nc.gpsimd.collective_compute(
                            "AllReduce", OP.add,
                            replica_groups=[[0, 1, 2, 3], [4, 5, 6, 7]],
                            ins=[ar_in[:].opt()], outs=[ar_out[:].opt()])

                    # ------- Stage E: delta + du + y_acc init -------
                    delta = [mp.tile([128, L], f32, tag=f"dl{j}",
                                     name=f"dl{j}") for j in range(2 * CT)]
                    du = [mp.tile([128, L], f32, tag=f"du{j}", name=f"du{j}")
                          for j in range(2 * CT)]
                    y_acc = [mp.tile([128, L], f32, tag=f"ya{j}",
                                     name=f"ya{j}") for j in range(2 * CT)]
                    if True:
                        for d in range(2):
                            dtv = mp.tile([DT_RANK, L], f32, tag="dtv",
                                          bufs=1, name=f"dtv{d}")
                            dtv2 = mp.tile([DT_RANK, L], f32, tag="dtv2",
                                           bufs=1, name=f"dtv2{d}")
                            nc.sync.dma_start(
                                dtv[:], ar_out[XD * d:XD * d + DT_RANK, :])
                            # single-writer ACT copy so the matmuls wait on
                            # one semaphore (the DMA fans out across queues)
                            nc.scalar.activation(dtv2[:], dtv[:], AF.Copy)
                            for i in range(CT):
                                for f in range(2):
                                    nc.tensor.matmul(
                                        ps_dt[:, 512 * f:512 * (f + 1)],
                                        dtw_t[d][:, 128 * i:128 * (i + 1)],
                                        dtv2[:, 512 * f:512 * (f + 1)],
                                        start=True, stop=True)
                                c = cst[d * CT + i]
                                j = d * CT + i
                                # softplus(x + b) = ln(1 + exp(x + b))
                                nc.scalar.activation(delta[j][:], ps_dt[:],
                                                     AF.Exp, bias=c[:, 5:6])
                                nc.scalar.activation(delta[j][:],
                                                     delta[j][:], AF.Ln,
                                                     bias=1.0)
                                nc.vector.tensor_mul(du[j][:], delta[j][:],
                                                     xt[j][:])
                                # init y accumulator with the xt*D skip term
                                nc.vector.scalar_tensor_tensor(
                                    y_acc[j][:], xt[j][:], c[:, 6:7],
                                    xt[j][:], op0=OP.mult, op1=OP.bypass)

                # ------- Stage F: selective scan -------
                with tc.tile_pool(name="stf", bufs=1) as fp:
                    bb0 = psG1.tile([128, L], f32, tag="bb", name="bb")
                    cb = psG1.tile([128, L], f32, tag="cb", name="cb")
                    # ping-pong the B-broadcast with the idle ps_dt banks so
                    # PE can fill state n+1 while DVE consumes state n
                    bb_pp = [bb0, ps_dt]
                    for d in range(2):
                        bcs = mp.tile([2 * D_STATE, L], f32, tag="bcs",
                                      bufs=1, name=f"bcs{d}")
                        bcf = mp.tile([2 * D_STATE, L], bf16, tag="bcf",
                                      bufs=1, name=f"bcf{d}")
                        nc.sync.dma_start(
                            bcs[:], ar_out[XD * d + DT_RANK:
                                           XD * d + DT_RANK + 2 * D_STATE, :])
                        nc.vector.tensor_copy(bcf[:], bcs[:])
                        for n in range(D_STATE):
                            bb = bb_pp[n % 2]
                            for f in range(2):
                                fs = slice(512 * f, 512 * (f + 1))
                                nc.tensor.matmul(
                                    bb[:, fs],
                                    selt[:, 128 * n:128 * (n + 1)],
                                    bcf[:, fs], start=True, stop=True)
                                nc.tensor.matmul(
                                    cb[:, fs],
                                    selt[:, 128 * (D_STATE + n):
                                          128 * (D_STATE + n + 1)],
                                    bcf[:, fs], start=True, stop=True)
                            cbs = fp.tile([128, L], f32, tag="cbs", bufs=2,
                                          name=f"cbs{d}_{n}")
                            nc.scalar.activation(cbs[:], cb[:], AF.Copy)
                            for i in range(CT):
                                j = d * CT + i
                                c = cst[j]
                                a_t = fp.tile([128, L], f32, tag="a_t",
                                              bufs=6, name=f"a{d}_{n}_{i}")
                                b_t = fp.tile([128, L], f32, tag="b_t",
                                              bufs=6, name=f"b{d}_{n}_{i}")
                                h_t = fp.tile([128, L], f32, tag="h_t",
                                              bufs=4, name=f"h{d}_{n}_{i}")
                                nc.vector.tensor_mul(b_t[:], du[j][:], bb[:])
                                nc.scalar.activation(
                                    a_t[:], delta[j][:], AF.Exp,
                                    scale=c[:, 7 + n:8 + n])
                                # zero the first decay on ACT (same engine as
                                # the exp) so the scan waits on one engine
                                nc.scalar.activation(a_t[:, 0:1],
                                                     delta[j][:, 0:1],
                                                     AF.Identity, scale=0.0)
                                nc.vector.tensor_tensor_scan(
                                    h_t[:], a_t[:], b_t[:], 0.0,
                                    op0=OP.mult, op1=OP.add)
                                if i == CT - 1:
                                    nc.gpsimd.tensor_mul(h_t[:], h_t[:],
                                                         cbs[:])
                                else:
                                    nc.vector.tensor_mul(h_t[:], h_t[:],
                                                         cb[:])
                                nc.gpsimd.tensor_add(y_acc[j][:],
                                                     y_acc[j][:], h_t[:])

                # ------- Stage G/H: combine directions + gate -------
                for i in range(CT):
                    nc.vector.tensor_add(y_acc[i][:], y_acc[i][:],
                                         rev_view(y_acc[CT + i][:]))
                    nc.vector.tensor_mul(y_acc[i][:], y_acc[i][:], sz[i][:])
                    touch(y_acc[i][:])

            # ---------- Stage I: out_proj partial ----------
            with tc.tile_pool(name="sti", bufs=1) as ip:
                for o in range(6):
                    for f in range(2):
                        for k in range(CT):
                            nc.tensor.matmul(
                                ps_dt[:, 512 * f:512 * (f + 1)],
                                wo_t[k][:, 128 * o:128 * (o + 1)],
                                y_acc[k][:, 512 * f:512 * (f + 1)],
                                start=(k == 0), stop=(k == CT - 1))
                    o_s = ip.tile([128, L], f32, tag="o_s", bufs=2,
                                  name=f"o_s{o}")
                    nc.scalar.activation(o_s[:], ps_dt[:], AF.Copy)
                    nc.sync.dma_start(outp[128 * o:128 * (o + 1), :], o_s[:])

    nc.compile()
    return nc


def _prep_inputs(inputs):
    import ml_dtypes

    h = np.asarray(inputs["hidden_states"], np.float32)
    w_in_full = np.asarray(inputs["in_proj_w"], np.float32)
    wo_full = np.asarray(inputs["out_proj_w"], np.float32)
    sel = np.zeros((2 * D_STATE, 2 * D_STATE * 128), np.float32)
    for n in range(2 * D_STATE):
        sel[n, 128 * n:128 * (n + 1)] = 1.0
    sel = sel.astype(ml_dtypes.bfloat16)

    pd = {}
    for d, tag in enumerate(["f", "r"]):
        pd[d] = dict(
            conv_w=np.asarray(inputs[f"conv_w_{tag}"], np.float32)[:, 0, :],
            conv_b=np.asarray(inputs[f"conv_b_{tag}"], np.float32),
            xp=np.asarray(inputs[f"x_proj_w_{tag}"], np.float32),
            dtw=np.asarray(inputs[f"dt_w_{tag}"], np.float32),
            dtb=np.asarray(inputs[f"dt_b_{tag}"], np.float32),
            A=-np.exp(np.asarray(inputs[f"A_log_{tag}"], np.float32)),
            D=np.asarray(inputs[f"D_{tag}"], np.float32),
        )

    in_maps = []
    for c in range(N_CORES):
        b, g = c // N_GROUPS, c % N_GROUPS
        sl = slice(CH * g, CH * (g + 1))
        m = {}
        m["hT"] = np.ascontiguousarray(h[b].T)
        rows = np.r_[CH * g:CH * (g + 1),
                     D_INNER + CH * g:D_INNER + CH * (g + 1)]
        m["w_in"] = np.ascontiguousarray(w_in_full[rows, :].T)
        cstm = np.zeros((2 * CH, 23), np.float32)
        xp_wm = np.zeros((2 * CH, 80), np.float32)
        dt_wm = np.zeros((2 * DT_RANK, CH), np.float32)
        for d in range(2):
            p = pd[d]
            r = slice(CH * d, CH * (d + 1))
            cstm[r, 0:4] = p["conv_w"][sl]
            cstm[r, 4] = p["conv_b"][sl]
            cstm[r, 5] = p["dtb"][sl]
            cstm[r, 6] = p["D"][sl]
            cstm[r, 7:23] = p["A"][sl]
            xp_wm[r, :] = p["xp"][:, sl].T
            dt_wm[DT_RANK * d:DT_RANK * (d + 1), :] = p["dtw"][sl, :].T
        m["consts"] = cstm
        m["xp_w"] = xp_wm
        m["dt_wT"] = dt_wm
        m["wo"] = np.ascontiguousarray(wo_full[:, sl].T)
        m["sel"] = sel
        in_maps.append(m)
    return in_maps


def kernel(**inputs):
    from concourse import bass_utils

    if "nc" not in _cache:
        _cache["nc"] = _build()
    nc = _cache["nc"]
    in_maps = _prep_inputs(inputs)
    res = bass_utils.run_bass_kernel_spmd(nc, in_maps,
                                          core_ids=list(range(N_CORES)))
    outs = [r["outp"] for r in res.results]
    full = np.empty((B, L, D_MODEL), np.float32)
    for b in range(B):
        acc = outs[b * N_GROUPS].astype(np.float64)
        for g in range(1, N_GROUPS):
            acc = acc + outs[b * N_GROUPS + g]
        full[b] = acc.T
    return full
